# revision 10
# baseline (speedup 1.0000x reference)
"""Trainium2 Bass kernel for nn_AblationRouter (moe_routing).

Computation (per batch row):
  h = EMA(x) with per-channel decay beta (constant 0.9 here)
  hid = relu([x, h] @ W1^T + b1);  route = hid @ W2^T + b2
  gates = softmax(route @ Wr^T + br)

Strategy: data-parallel over B=8 batch rows, one per NeuronCore.
The EMA recurrence is computed as matmuls against a precomputed decay
matrix over 512-token blocks with a 128-token lookback (beta^128 ~ 1e-6,
negligible vs matmul rounding -> no serial carry chain).  All activations
are kept channel-major ([channel, token]) so the three matmuls chain on
the TensorEngine without transposes.  The EMA always runs in float32r;
the MLP/router matmuls run in MM_DTYPE (bf16 or float32r).  b2 is folded
into an effective router bias: logits = route@Wr^T + (br + Wr@b2).
Weights are streamed per window of BPW 512-token blocks (bf16 fits
2-block windows in SBUF -> half the weight re-streaming).
"""

import sys

if "/opt/trn_rl_repo" not in sys.path:
    sys.path.insert(0, "/opt/trn_rl_repo")

import numpy as np
import ml_dtypes

# Problem shapes (hardcoded per harness contract)
B, T, D, E, H = 8, 2048, 1024, 64, 4096
TP = T  # padded token count (2047 real + 1 pad)
BLK = 512  # token block (psum free-dim)
LB = 128  # lookback tokens
SC = (LB + BLK) // 128  # 5 s-chunks per block
NBLK = TP // BLK  # 4 blocks
NDT = D // 128  # 8 d-tiles
NHT = H // 128  # 32 hid-tiles
NKT = (2 * D) // 128  # 16 k-tiles for M1

MM_DEFAULT = "f32r"  # "bf16" | "f32r"


def _build_program(mm=MM_DEFAULT):
    import concourse.bacc as bacc
    import concourse.mybir as mybir
    import concourse.tile as tile
    from concourse._compat import axon_active

    f32 = mybir.dt.float32
    f32r = mybir.dt.float32r
    bf16 = mybir.dt.bfloat16
    AF = mybir.ActivationFunctionType
    AX = mybir.AxisListType
    ALU = mybir.AluOpType

    wdt = bf16 if mm == "bf16" else f32r
    BPW = 2 if mm == "bf16" else 1  # blocks per weight-streaming window
    NW = NBLK // BPW
    WTOK = BPW * BLK  # tokens per window

    nc = bacc.Bacc("TRN2", target_bir_lowering=False, debug=not axon_active())

    # --- DRAM I/O ---
    x_pad = nc.dram_tensor("x_pad", [LB + TP, D], f32r, kind="ExternalInput")
    xT = nc.dram_tensor("xT", [D, TP], wdt, kind="ExternalInput")
    mdec = nc.dram_tensor("mdec", [SC * 128, BLK], f32r, kind="ExternalInput")
    # weight tiles pre-arranged so one (128 part x long contiguous row) DMA
    # loads a full stationary-tile group
    w1t = nc.dram_tensor("w1t", [NHT, 128, NKT * 128], wdt, kind="ExternalInput")
    w2t = nc.dram_tensor("w2t", [NDT, 128, NHT * 128], wdt, kind="ExternalInput")
    b1t = nc.dram_tensor("b1t", [128, NHT], f32, kind="ExternalInput")
    wrt = nc.dram_tensor("wrt", [128, NDT * E], wdt, kind="ExternalInput")
    brb = nc.dram_tensor("brb", [128, E], f32, kind="ExternalInput")
    out = nc.dram_tensor("out", [TP, E], f32, kind="ExternalOutput")

    with tile.TileContext(nc) as tc:
        with (
            tc.tile_pool(name="const", bufs=1) as cpool,
            tc.tile_pool(name="xin", bufs=1) as xpool,
            tc.tile_pool(name="xtin", bufs=1) as xtpool,
            tc.tile_pool(name="acts", bufs=1) as apool,
            tc.tile_pool(name="w1", bufs=3) as w1pool,
            tc.tile_pool(name="w2", bufs=2) as w2pool,
            tc.tile_pool(name="sm", bufs=2) as smpool,
            tc.tile_pool(name="ema_ps", bufs=2, space="PSUM") as ema_ps,
            tc.tile_pool(name="m1_ps", bufs=2, space="PSUM") as m1_ps,
            tc.tile_pool(name="m2_ps", bufs=2, space="PSUM") as m2_ps,
            tc.tile_pool(name="m3_ps", bufs=2, space="PSUM") as m3_ps,
        ):
            # Constants resident in SBUF
            m_sb = cpool.tile([128, SC * BLK], f32r, tag="mdec")
            for sc in range(SC):
                nc.sync.dma_start(
                    m_sb[:, sc * BLK : (sc + 1) * BLK],
                    mdec[sc * 128 : (sc + 1) * 128, :],
                )
            b1_sb = cpool.tile([128, NHT], f32, tag="b1")
            nc.sync.dma_start(b1_sb[:], b1t[:])
            wr_sb = cpool.tile([128, NDT * E], wdt, tag="wr")
            nc.sync.dma_start(wr_sb[:], wrt[:])
            br_sb = cpool.tile([128, E], f32, tag="br")
            nc.sync.dma_start(br_sb[:], brb[:])

            NSR = LB // 128 + BPW * (BLK // 128)  # x rows (128-chunks) per window
            for w in range(NW):
                t0 = w * WTOK
                # --- stream inputs for this window ---
                x_sb = xpool.tile([128, NSR * D], f32r, tag="x")
                for sc in range(NSR):
                    nc.sync.dma_start(
                        x_sb[:, sc * D : (sc + 1) * D],
                        x_pad[t0 + sc * 128 : t0 + (sc + 1) * 128, :],
                    )
                xt_sb = xtpool.tile([128, NDT * WTOK], wdt, tag="xt")
                for dt in range(NDT):
                    nc.sync.dma_start(
                        xt_sb[:, dt * WTOK : (dt + 1) * WTOK],
                        xT[dt * 128 : (dt + 1) * 128, t0 : t0 + WTOK],
                    )

                # --- EMA -> hT (channel-major) ---
                ht_sb = apool.tile([128, NDT * WTOK], wdt, tag="ht")
                for blk in range(BPW):
                    for dt in range(NDT):
                        ps = ema_ps.tile([128, BLK], f32, tag="ema")
                        for sc in range(SC):
                            off = (blk * (BLK // 128) + sc) * D
                            nc.tensor.matmul(
                                ps[:],
                                x_sb[:, off + dt * 128 : off + (dt + 1) * 128],
                                m_sb[:, sc * BLK : (sc + 1) * BLK],
                                start=(sc == 0),
                                stop=(sc == SC - 1),
                            )
                        nc.vector.tensor_copy(
                            ht_sb[:, dt * WTOK + blk * BLK : dt * WTOK + (blk + 1) * BLK],
                            ps[:],
                        )

                # --- M1: hidT = relu(W1T.T @ inpT + b1) ---
                hid_sb = apool.tile([128, NHT * WTOK], wdt, tag="hid")
                for ht in range(NHT):
                    w1_sb = w1pool.tile([128, NKT * 128], wdt, tag="w1")
                    half = NKT * 64
                    nc.sync.dma_start(w1_sb[:, :half], w1t[ht, :, :half])
                    nc.sync.dma_start(w1_sb[:, half:], w1t[ht, :, half:])
                    for blk in range(BPW):
                        ps1 = m1_ps.tile([128, BLK], f32, tag="m1")
                        for kt in range(NKT):
                            src = xt_sb if kt < NDT else ht_sb
                            doff = (kt % NDT) * WTOK + blk * BLK
                            nc.tensor.matmul(
                                ps1[:],
                                w1_sb[:, kt * 128 : (kt + 1) * 128],
                                src[:, doff : doff + BLK],
                                start=(kt == 0),
                                stop=(kt == NKT - 1),
                            )
                        dst = hid_sb[
                            :, ht * WTOK + blk * BLK : ht * WTOK + (blk + 1) * BLK
                        ]
                        if mm == "bf16":
                            nc.scalar.activation(
                                dst, ps1[:], AF.Relu, bias=b1_sb[:, ht : ht + 1]
                            )
                        else:
                            # relu(x + b1) in one DVE op, rounding to f32r
                            nc.vector.tensor_scalar(
                                dst,
                                ps1[:],
                                b1_sb[:, ht : ht + 1],
                                0.0,
                                op0=ALU.add,
                                op1=ALU.max,
                            )

                # --- M2: routeT = W2T.T @ hidT (b2 folded into br_eff) ---
                rt_sb = apool.tile([128, NDT * WTOK], wdt, tag="route")
                for dt in range(NDT):
                    w2_sb = w2pool.tile([128, NHT * 128], wdt, tag="w2")
                    half = NHT * 64
                    nc.sync.dma_start(w2_sb[:, :half], w2t[dt, :, :half])
                    nc.sync.dma_start(w2_sb[:, half:], w2t[dt, :, half:])
                    for blk in range(BPW):
                        ps2 = m2_ps.tile([128, BLK], f32, tag="m2")
                        for ht in range(NHT):
                            nc.tensor.matmul(
                                ps2[:],
                                w2_sb[:, ht * 128 : (ht + 1) * 128],
                                hid_sb[
                                    :,
                                    ht * WTOK + blk * BLK : ht * WTOK + (blk + 1) * BLK,
                                ],
                                start=(ht == 0),
                                stop=(ht == NHT - 1),
                            )
                        nc.vector.tensor_copy(
                            rt_sb[:, dt * WTOK + blk * BLK : dt * WTOK + (blk + 1) * BLK],
                            ps2[:],
                        )

                # --- M3 + softmax (tokens on partitions) ---
                for blk in range(BPW):
                    bt0 = t0 + blk * BLK
                    ot = smpool.tile([128, (BLK // 128) * E], f32, tag="ot")
                    for tt in range(BLK // 128):
                        ps3 = m3_ps.tile([128, E], f32, tag="m3")
                        for dt in range(NDT):
                            roff = dt * WTOK + blk * BLK + tt * 128
                            nc.tensor.matmul(
                                ps3[:],
                                rt_sb[:, roff : roff + 128],
                                wr_sb[:, dt * E : (dt + 1) * E],
                                start=(dt == 0),
                                stop=(dt == NDT - 1),
                            )
                        lg = smpool.tile([128, E], f32, tag="lg")
                        nc.vector.tensor_add(lg[:], ps3[:], br_sb[:])
                        negm = smpool.tile([128, 1], f32, tag="negm")
                        nc.vector.reduce_max(negm[:], lg[:], axis=AX.X, negate=True)
                        ex = smpool.tile([128, E], f32, tag="ex")
                        nc.scalar.activation(ex[:], lg[:], AF.Exp, bias=negm[:])
                        ssum = smpool.tile([128, 1], f32, tag="ssum")
                        nc.vector.reduce_sum(ssum[:], ex[:], axis=AX.X)
                        rcp = smpool.tile([128, 1], f32, tag="rcp")
                        nc.vector.reciprocal(rcp[:], ssum[:])
                        nc.vector.tensor_scalar_mul(
                            ot[:, tt * E : (tt + 1) * E], ex[:], rcp[:]
                        )
                    # single DMA per block: [4 tok-tiles, 128, E]
                    nc.sync.dma_start(
                        out[bt0 : bt0 + BLK, :].rearrange("(tt p) e -> p tt e", p=128),
                        ot[:].rearrange("p (tt e) -> p tt e", e=E),
                    )

    nc.compile()
    return nc


_prepared = {}


def _prepare_host_inputs(seq, beta_raw, W1, b1, W2, b2, Wr, br, mm=MM_DEFAULT):
    np_wdt = ml_dtypes.bfloat16 if mm == "bf16" else np.float32
    seq = np.asarray(seq, np.float32)
    beta = 1.0 / (1.0 + np.exp(-np.asarray(beta_raw, np.float64)))
    assert beta.max() - beta.min() < 1e-6, "kernel assumes channel-constant beta"
    b = float(beta[0])
    assert b ** LB < 1e-4, "lookback too short for this beta"

    x = seq[:, : T - 1, :]  # [B, 2047, D]

    # decay matrix: mdec[s, t] = b^((t+LB)-s) for (t+LB)>=s else 0
    s_idx = np.arange(LB + BLK)[:, None]
    t_idx = np.arange(BLK)[None, :]
    expo = (t_idx + LB) - s_idx
    mdec = np.where(expo >= 0, b ** np.maximum(expo, 0), 0.0).astype(np.float32)

    W1 = np.asarray(W1, np.float32)
    W2 = np.asarray(W2, np.float32)
    Wr = np.asarray(Wr, np.float32)
    # w1t[ht, p, kt*128+j] = W1[ht*128+j, kt*128+p] -> SBUF partition rows are
    # one contiguous DMA line per stationary-tile group
    w1t = np.ascontiguousarray(
        W1.reshape(NHT, 128, NKT, 128).transpose(0, 3, 2, 1).reshape(NHT, 128, NKT * 128)
    ).astype(np_wdt)
    w2t = np.ascontiguousarray(
        W2.reshape(NDT, 128, NHT, 128).transpose(0, 3, 2, 1).reshape(NDT, 128, NHT * 128)
    ).astype(np_wdt)
    b1t = np.ascontiguousarray(np.asarray(b1, np.float32).reshape(NHT, 128).T)
    wrt = np.ascontiguousarray(
        Wr.T.reshape(NDT, 128, E).transpose(1, 0, 2).reshape(128, NDT * E)
    ).astype(np_wdt)
    # fold b2 into router bias: logits = route@Wr^T + (br + Wr@b2)
    br_eff = np.asarray(br, np.float32) + Wr @ np.asarray(b2, np.float32)
    brb = np.ascontiguousarray(np.tile(br_eff[None, :], (128, 1)))

    shared = dict(mdec=mdec, w1t=w1t, w2t=w2t, b1t=b1t, wrt=wrt, brb=brb)
    in_maps = []
    for bi in range(B):
        x_pad = np.zeros((LB + TP, D), np.float32)
        x_pad[LB : LB + T - 1] = x[bi]
        xT = np.zeros((D, TP), np.float32)
        xT[:, : T - 1] = x[bi].T
        m = dict(shared)
        m["x_pad"] = x_pad
        m["xT"] = np.ascontiguousarray(xT).astype(np_wdt)
        in_maps.append(m)
    return in_maps


def kernel(**inputs):
    from concourse import bass_utils

    if "nc" not in _prepared:
        _prepared["nc"] = _build_program()
    nc = _prepared["nc"]
    in_maps = _prepare_host_inputs(**inputs)
    res = bass_utils.run_bass_kernel_spmd(nc, in_maps, core_ids=list(range(B)))
    outs = np.stack([r["out"] for r in res.results], axis=0)  # [B, TP, E]
    return outs[:, : T - 1, :].astype(np.float32)
